# revision 18
# baseline (speedup 1.0000x reference)
"""Multi-head attention (B=4, S=2048, E=1024, H=16, hd=64) on 8 TRN2 cores.

Sharding: core c -> batch b = c//2, head-half hh = c%2 (8 heads = 512 internal
dims).  Data parallel on B, tensor parallel on heads.  Each core computes a
partial out-projection for its batch; the host sums the two head-half partials
per batch and adds the (folded) output bias.

Device dataflow (bf16 matmuls, fp32 PSUM accumulation):
  - host pre-transposes q/k/v to (E, S), casts to bf16, and splits v/q into
    contiguous 512-seq-col quarters so the DMA stream delivers exactly what
    the next projection chain needs (strided column slices of a wide array
    DMA pathologically slowly; contiguous quarters stream at full rate).
  - DMA priority: wv, v quarters, wq, q quarter 0, wk, k, q quarters 1-3, wo.
    vh chains pipeline with the v quarters; attention starts as soon as k
    lands (~46us) instead of after the whole input set.
  - a block of dependency-free junk matmuls at the very start holds the PE's
    HAM activity window busy so the real chains run at 2.4 GHz, not the cold
    1.2 GHz default.
  - attention per head-PAIR (2g, 2g+1) per 512-query chunk: row-group
    concurrent K=64 scoresT matmuls for both heads into one PSUM tile, one
    Exp over both (scale 1/8 pre-folded into Wq on host), then M=65 AV
    matmuls whose ones-column accumulates the softmax denominator in row 64.
  - ScalarE runs ONLY the Exps in steady state (the 1104ns/kt pacer); all
    projection PSUM evacuations ride DVE (tensor_scalar_add with the bias
    column).  The two head m=0 evacs use the then-idle ScalarE.
  - division: AV promptly evacuated PSUM->SBUF (frees the accumulator),
    denominator row DMA round-trips through DRAM to broadcast across
    partitions, DVE fast-reciprocal + multiply into attn_outT -- exactly the
    lhsT needed for the out-projection po (q x E) = attn_outT^T @ Wo_loc.
    The final (g3,qc3) division runs in two column halves so the last
    out-projection can start earlier, with junk warm-keeper matmuls holding
    the PE clock up through the division latency.
  - engines run their streams in order, so projection m-tile g+1 matmuls are
    explicitly interleaved into attention pair g's steps (and out-projection
    into pair 3's) to keep TensorE busy while ScalarE paces the exps.
  - output stored fp16 (host sums the two head-half partials in fp32).
"""

import math
import sys
from contextlib import ExitStack

sys.path.insert(0, "/opt/trn_rl_repo")

import numpy as np
import ml_dtypes

import concourse.bass as bass
from concourse import bacc
import concourse.mybir as mybir
import concourse.tile as tile

F32 = mybir.dt.float32
F16 = mybir.dt.float16
BF16 = mybir.dt.bfloat16
AF = mybir.ActivationFunctionType
ALU = mybir.AluOpType

B, S, E = 4, 2048, 1024
H, HD = 16, 64
HLOC = 8          # heads per core
ILOC = HLOC * HD  # 512 internal dims per core
KT = E // 128     # 8 embed k-tiles
ST = S // 128     # 16 seq tiles
NCORES = 8
SCALE = 1.0 / math.sqrt(HD)  # 1/8


def build_nc():
    nc = bacc.Bacc()

    vq_d = [nc.declare_dram_parameter(f"vq{j}", [E, 512], BF16,
                                      isOutput=False).ap() for j in range(4)]
    qq_d = [nc.declare_dram_parameter(f"qq{j}", [E, 512], BF16,
                                      isOutput=False).ap() for j in range(4)]
    kT_d = nc.declare_dram_parameter("kT", [E, S], BF16, isOutput=False).ap()
    wq_d = nc.declare_dram_parameter("wq", [E, ILOC], BF16, isOutput=False).ap()
    wk_d = nc.declare_dram_parameter("wk", [E, ILOC], BF16, isOutput=False).ap()
    wv_d = nc.declare_dram_parameter("wv", [E, ILOC], BF16, isOutput=False).ap()
    wo_d = nc.declare_dram_parameter("wo", [ILOC, E], BF16, isOutput=False).ap()
    bq_d = nc.declare_dram_parameter("bq", [128, 4], F32, isOutput=False).ap()
    bk_d = nc.declare_dram_parameter("bk", [128, 4], F32, isOutput=False).ap()
    out_d = nc.declare_dram_parameter("out", [S, E], F16, isOutput=True).ap()
    dscr = nc.dram_tensor("dscratch", [48, 512], F32).ap()

    with tile.TileContext(nc) as tc, ExitStack() as ctx:
        # ---- pools (PSUM: pp 2x1 + sc 2x2 + av 2x1 = 8 banks) ----
        psum = ctx.enter_context(tc.tile_pool(name="psum", bufs=2, space="PSUM"))
        av_pool = ctx.enter_context(tc.tile_pool(name="avp", bufs=2, space="PSUM"))
        qhT_pool = ctx.enter_context(tc.tile_pool(name="qhT", bufs=2))
        khT_pool = ctx.enter_context(tc.tile_pool(name="khT", bufs=2))
        vh_pool = ctx.enter_context(tc.tile_pool(name="vh", bufs=ST))
        bias_pool = ctx.enter_context(tc.tile_pool(name="bias", bufs=1))
        wpool = ctx.enter_context(tc.tile_pool(name="w_in", bufs=4))
        # svq: v quarters [128, KT, 512]; the q quarters 1-3 reuse the same
        # 4-slot ring once vh has consumed the corresponding v quarter.
        svq_pool = ctx.enter_context(tc.tile_pool(name="svq", bufs=4))
        stage_pool = ctx.enter_context(tc.tile_pool(name="stage", bufs=8))
        exp_pool = ctx.enter_context(tc.tile_pool(name="exp", bufs=4))
        attnT_pool = ctx.enter_context(tc.tile_pool(name="attnT", bufs=4))
        small_pool = ctx.enter_context(tc.tile_pool(name="small", bufs=2))
        bc_pool = ctx.enter_context(tc.tile_pool(name="bcb", bufs=2))
        tmp_pool = ctx.enter_context(tc.tile_pool(name="tmpp", bufs=1))
        out_pool = ctx.enter_context(tc.tile_pool(name="outbuf", bufs=2))

        # qhT/khT for head-pair m are only live while pair m-1/m run, so a
        # 2-slot ring suffices; tiles are created lazily in emission order
        # (m projected as fillers during pair m-1).
        qhT_tiles, khT_tiles = {}, {}

        def qhT(m):
            if m not in qhT_tiles:
                qhT_tiles[m] = qhT_pool.tile(
                    [128, S], BF16, tag="qhT", name=f"qhT{m}")
            return qhT_tiles[m]

        def khT(m):
            if m not in khT_tiles:
                khT_tiles[m] = khT_pool.tile(
                    [128, S], BF16, tag="khT", name=f"khT{m}")
            return khT_tiles[m]

        vh = [vh_pool.tile([128, HLOC * 65], BF16, tag="vh", name=f"vh{i}")
              for i in range(ST)]

        bq_t = bias_pool.tile([128, 4], F32, tag="bq")
        bk_t = bias_pool.tile([128, 4], F32, tag="bk")
        nc.sync.dma_start(bq_t[:], bq_d[:])
        nc.sync.dma_start(bk_t[:], bk_d[:])

        wq_t = wpool.tile([128, KT, ILOC], BF16, tag="w")
        wk_t = wpool.tile([128, KT, ILOC], BF16, tag="w")
        wv_t = wpool.tile([128, KT, ILOC], BF16, tag="w")
        wo_t = wpool.tile([128, 4, E], BF16, tag="w")
        sq0_t = wpool.tile([128, KT, 512], BF16, tag="sq0")
        warm_src = wpool.tile([128, 512], BF16, tag="warm")
        attnT = [attnT_pool.tile([128, S], BF16, tag="attnT",
                                 name=f"attnT{i}") for i in range(4)]

        # ---- stage loads in DMA priority order ----
        nc.sync.dma_start(wv_t[:], wv_d.rearrange("(k p) n -> p k n", p=128))
        svq = []
        for j in range(4):
            t = svq_pool.tile([128, KT, 512], BF16, tag="svq", name=f"svq{j}")
            nc.sync.dma_start(t[:], vq_d[j].rearrange("(k p) n -> p k n",
                                                      p=128))
            svq.append(t)
        nc.sync.dma_start(wq_t[:], wq_d.rearrange("(k p) n -> p k n", p=128))
        nc.sync.dma_start(sq0_t[:], qq_d[0].rearrange("(k p) n -> p k n",
                                                      p=128))
        nc.sync.dma_start(wk_t[:], wk_d.rearrange("(k p) n -> p k n", p=128))
        stg_k = []
        for kk in range(KT):
            t = stage_pool.tile([128, S], BF16, tag="stage", name=f"sk{kk}")
            nc.sync.dma_start(t[:], kT_d[kk * 128:(kk + 1) * 128, :])
            stg_k.append(t)
        sq123 = []
        for j in range(1, 4):
            t = svq_pool.tile([128, KT, 512], BF16, tag="svq", name=f"sq{j}")
            nc.sync.dma_start(t[:], qq_d[j].rearrange("(k p) n -> p k n",
                                                      p=128))
            sq123.append(t)
        nc.sync.dma_start(wo_t[:], wo_d.rearrange("(i p) n -> p i n", p=128))

        # ---- PE warm-up: junk matmuls with no data deps hold the HAM
        # activity window busy so the projection chains run at 2.4 GHz ----
        nc.vector.memset(warm_src[:], 0.0)
        for _ in range(32):
            wps = psum.tile([128, 512], F32, tag="pp", name="wps")
            nc.tensor.matmul(wps[:], lhsT=warm_src[:, 0:128],
                             rhs=warm_src[:], start=True, stop=True)

        def vh_chain(st):
            ps = psum.tile([128, 512], F32, tag="pp", name="psv")
            src = svq[st // 4]
            cs = (st % 4) * 128
            for kk in range(KT):
                nc.tensor.matmul(
                    ps[:],
                    lhsT=src[:, kk, cs:cs + 128],
                    rhs=wv_t[:, kk, :],
                    start=(kk == 0), stop=(kk == KT - 1),
                )
            pin = ps[:].rearrange("p (a b x) -> p a b x", b=2, x=64)
            pout = vh[st].rearrange("p (a c) -> p a c", c=130)
            nc.vector.tensor_copy(pout[:, :, 0:64], pin[:, :, 0, :])
            nc.vector.tensor_copy(pout[:, :, 65:129], pin[:, :, 1, :])
            ones = vh[st].rearrange("p (h x) -> p h x", x=65)[:, :, 64:65]
            nc.vector.memset(ones, 1.0)

        def k_src(kk, quarter):
            return stg_k[kk][:, quarter * 512:(quarter + 1) * 512]

        def q_src(kk, quarter):
            if quarter == 0:
                return sq0_t[:, kk, :]
            return sq123[quarter - 1][:, kk, :]

        def proj_ops(m, which="kq", quarters=range(4), evac_scalar=False):
            """Closure list projecting khT/qhT m-tile chains."""
            ops = []
            srcs = {"k": ((k_src, wk_t, khT, bk_t),),
                    "q": ((q_src, wq_t, qhT, bq_t),)}
            chosen = srcs["k"] + srcs["q"] if which == "kq" else srcs[which]
            for src_fn, w_t, dst, b_t in chosen:
                for quarter in quarters:
                    cols = slice(quarter * 512, (quarter + 1) * 512)
                    holder = {}

                    for kk in range(KT):
                        def mm(kk=kk, src_fn=src_fn, w_t=w_t, quarter=quarter,
                               holder=holder, first=(kk == 0)):
                            if first:
                                holder["ps"] = psum.tile(
                                    [128, 512], F32, tag="pp", name="psqk")
                            nc.tensor.matmul(
                                holder["ps"][:],
                                lhsT=w_t[:, kk, m * 128:(m + 1) * 128],
                                rhs=src_fn(kk, quarter),
                                start=(kk == 0), stop=(kk == KT - 1),
                            )
                        ops.append(mm)

                    def evac(dst=dst, cols=cols, b_t=b_t, m=m, holder=holder,
                             evac_scalar=evac_scalar):
                        if evac_scalar:
                            nc.scalar.activation(
                                dst(m)[:, cols], holder["ps"][:], AF.Identity,
                                bias=b_t[:, m:m + 1],
                            )
                        else:
                            nc.vector.tensor_scalar_add(
                                dst(m)[:, cols], holder["ps"][:],
                                b_t[:, m:m + 1],
                            )
                    ops.append(evac)
            return ops

        def outproj_ops(qc):
            """Closure list projecting output for query chunk qc."""
            ops = []
            for qt in range(qc * 4, qc * 4 + 4):
                holder = {}

                for c in range(2):
                    for it in range(4):
                        def mm(qt=qt, c=c, it=it, holder=holder,
                               first=(c == 0 and it == 0)):
                            if first:
                                holder["ot"] = out_pool.tile(
                                    [128, 1024], F16, tag="ot", name="ot")
                            if it == 0:
                                holder["po"] = psum.tile(
                                    [128, 512], F32, tag="pp", name="po")
                            nc.tensor.matmul(
                                holder["po"][:],
                                lhsT=attnT[it][:, qt * 128:(qt + 1) * 128],
                                rhs=wo_t[:, it, c * 512:(c + 1) * 512],
                                start=(it == 0), stop=(it == 3),
                            )
                        ops.append(mm)

                    def evac(qt=qt, c=c, holder=holder, last=(c == 1)):
                        nc.vector.tensor_copy(
                            holder["ot"][:, c * 512:(c + 1) * 512],
                            holder["po"][:])
                        if last:
                            nc.sync.dma_start(
                                out_d[qt * 128:(qt + 1) * 128, :],
                                holder["ot"][:])
                    ops.append(evac)
            return ops

        def division(g, qc, avA, avB, halves=1):
            """Softmax division for one (g, qc): evacuate av, broadcast the
            denominator row via DRAM, reciprocal + multiply into attnT."""
            qcols0 = qc * 512
            avsA = small_pool.tile([65, 512], F32, tag="avs", name="avsA")
            avsB = small_pool.tile([65, 512], F32, tag="avs", name="avsB")
            nc.vector.tensor_copy(avsA[:], avA[:])
            nc.vector.tensor_copy(avsB[:], avB[:])
            w = 512 // halves
            for h in range(halves):
                cs = h * w
                idx = 32 + 2 * h if halves > 1 else (g * 4 + qc) * 2
                nc.gpsimd.dma_start(dscr[idx:idx + 1, 0:w],
                                    avsA[64:65, cs:cs + w])
                nc.gpsimd.dma_start(dscr[idx + 1:idx + 2, 0:w],
                                    avsB[64:65, cs:cs + w])
                bcA = bc_pool.tile([64, 512], F32, tag="bc", name="bcA")
                bcB = bc_pool.tile([64, 512], F32, tag="bc", name="bcB")
                nc.gpsimd.dma_start(
                    bcA[:, 0:w].rearrange("p (o n) -> p o n", o=1),
                    dscr[idx, 0:w].partition_broadcast(64))
                nc.gpsimd.dma_start(
                    bcB[:, 0:w].rearrange("p (o n) -> p o n", o=1),
                    dscr[idx + 1, 0:w].partition_broadcast(64))
                nc.vector.reciprocal_approx_fast(bcA[:, 0:w], bcA[:, 0:w])
                nc.vector.reciprocal_approx_fast(bcB[:, 0:w], bcB[:, 0:w])
                nc.vector.tensor_mul(
                    attnT[g][0:64, qcols0 + cs:qcols0 + cs + w],
                    avsA[0:64, cs:cs + w], bcA[:, 0:w])
                tmp = tmp_pool.tile([64, 512], BF16, tag="tmp", name="tmp")
                nc.vector.tensor_mul(tmp[:, 0:w],
                                     avsB[0:64, cs:cs + w], bcB[:, 0:w])
                nc.gpsimd.dma_start(
                    attnT[g][64:128, qcols0 + cs:qcols0 + cs + w],
                    tmp[:, 0:w])

        # ---- head: vh chains pipeline with the v-quarter DMAs, then the
        # m=0 chunk-0 projections (evacs on the then-idle ScalarE) ----
        for st in range(ST):
            vh_chain(st)
        for op in proj_ops(0, "q", quarters=(0,), evac_scalar=True):
            op()
        for op in proj_ops(0, "k", quarters=(0,), evac_scalar=True):
            op()

        for g in range(4):              # head pair (2g, 2g+1)
            hA, hB = 2 * g, 2 * g + 1
            if g == 0:
                fillers = (proj_ops(0, "k", quarters=(1, 2, 3))
                           + proj_ops(0, "q", quarters=(1, 2, 3))
                           + proj_ops(1))
            elif g < 3:
                fillers = proj_ops(g + 1)
            else:
                fillers = []
            steps_left = 4 * ST
            for qc in range(4):         # 512-query chunks
                if g == 3 and qc >= 1:
                    fillers.extend(outproj_ops(qc - 1))
                qcols = slice(qc * 512, (qc + 1) * 512)
                avA = av_pool.tile([65, 512], F32, tag="av", name="avA")
                avB = av_pool.tile([65, 512], F32, tag="av", name="avB")
                for kt in range(ST):
                    sc = psum.tile([128, 1024], F32, tag="sc", name="sc")
                    nc.tensor.matmul(
                        sc[:, 0:512],
                        lhsT=khT(g)[0:64, kt * 128:(kt + 1) * 128],
                        rhs=qhT(g)[0:64, qcols],
                        start=True, stop=True,
                    )
                    nc.tensor.matmul(
                        sc[:, 512:1024],
                        lhsT=khT(g)[64:128, kt * 128:(kt + 1) * 128],
                        rhs=qhT(g)[64:128, qcols],
                        start=True, stop=True,
                    )
                    ex = exp_pool.tile([128, 1024], BF16, tag="exp", name="ex")
                    nc.scalar.activation(ex[:], sc[:], AF.Exp)
                    first, last = (kt == 0), (kt == ST - 1)
                    nc.tensor.matmul(
                        avA[0:65, :],
                        lhsT=vh[kt][:, hA * 65:hA * 65 + 65],
                        rhs=ex[:, 0:512],
                        start=first, stop=last,
                    )
                    nc.tensor.matmul(
                        avB[0:65, :],
                        lhsT=vh[kt][:, hB * 65:hB * 65 + 65],
                        rhs=ex[:, 512:1024],
                        start=first, stop=last,
                    )
                    # pace interleaved filler work (proj / out-proj);
                    # front-load during (g0,qc0) so khT/qhT m=0 quarters
                    # finish before the kt sweep reaches them
                    steps_left -= 1
                    n_take = -(-len(fillers) // max(steps_left, 1)) \
                        if fillers else 0
                    if g == 0 and qc == 0 and fillers:
                        n_take = max(n_take, 3)
                    for _ in range(min(n_take, len(fillers))):
                        fillers.pop(0)()
                if g == 3 and qc == 3:
                    # hold the PE warm through the final division latency
                    for _ in range(20):
                        wps = psum.tile([128, 512], F32, tag="pp", name="wk2")
                        nc.tensor.matmul(wps[:], lhsT=warm_src[:, 0:128],
                                         rhs=warm_src[:], start=True,
                                         stop=True)
                    division(g, qc, avA, avB, halves=4)
                else:
                    division(g, qc, avA, avB)
            # flush any leftover fillers for this pair
            for op in fillers:
                op()

        # final out-projection chunk
        for op in outproj_ops(3):
            op()

    nc.finalize()
    return nc


def make_in_maps(q, k, v, Wq, bq, Wk, bk, Wv, bv, Wo, bo):
    """Per-core input dicts + the folded host-side bias."""
    bf = ml_dtypes.bfloat16
    qT = [np.ascontiguousarray(q[b].T).astype(bf) for b in range(B)]
    kT = [np.ascontiguousarray(k[b].T).astype(bf) for b in range(B)]
    vT = [np.ascontiguousarray(v[b].T).astype(bf) for b in range(B)]
    in_maps = []
    for c in range(NCORES):
        b, hh = divmod(c, 2)
        isl = slice(hh * ILOC, (hh + 1) * ILOC)
        bq_loc = np.ascontiguousarray(
            (bq[isl] * SCALE).reshape(4, 128).T).astype(np.float32)
        bk_loc = np.ascontiguousarray(
            bk[isl].reshape(4, 128).T).astype(np.float32)
        m = {
            "kT": kT[b],
            "wq": np.ascontiguousarray(Wq[:, isl] * SCALE).astype(bf),
            "wk": np.ascontiguousarray(Wk[:, isl]).astype(bf),
            "wv": np.ascontiguousarray(Wv[:, isl]).astype(bf),
            "wo": np.ascontiguousarray(Wo[isl, :]).astype(bf),
            "bq": bq_loc, "bk": bk_loc,
        }
        for j in range(4):
            m[f"vq{j}"] = np.ascontiguousarray(vT[b][:, j * 512:(j + 1) * 512])
            m[f"qq{j}"] = np.ascontiguousarray(qT[b][:, j * 512:(j + 1) * 512])
        in_maps.append(m)
    bo_eff = (bo + bv @ Wo).astype(np.float32)
    return in_maps, bo_eff


_NC_CACHE = None


def kernel(q, k, v, Wq, bq, Wk, bk, Wv, bv, Wo, bo):
    global _NC_CACHE
    from concourse.bass_utils import run_bass_kernel_spmd

    if _NC_CACHE is None:
        _NC_CACHE = build_nc()
    nc = _NC_CACHE
    in_maps, bo_eff = make_in_maps(q, k, v, Wq, bq, Wk, bk, Wv, bv, Wo, bo)
    res = run_bass_kernel_spmd(nc, in_maps, list(range(NCORES)))
    out = np.empty((B, S, E), np.float32)
    for b in range(B):
        out[b] = (res.results[2 * b]["out"].astype(np.float32)
                  + res.results[2 * b + 1]["out"].astype(np.float32)
                  + bo_eff)
    return out


# revision 19
# speedup vs baseline: 1.2240x; 1.2240x over previous
"""Multi-head attention (B=4, S=2048, E=1024, H=16, hd=64) on 8 TRN2 cores.

Sharding: core c -> batch b = c//2, head-half hh = c%2 (8 heads = 512 internal
dims).  Data parallel on B, tensor parallel on heads.  Each core computes a
partial out-projection for its batch; the host sums the two head-half partials
per batch and adds the (folded) output bias.

Device dataflow (bf16 matmuls, fp32 PSUM accumulation):
  - host pre-transposes q/k/v to (E, S), casts to bf16, and splits v/q into
    contiguous 512-seq-col quarters so the DMA stream delivers exactly what
    the next projection chain needs (strided column slices of a wide array
    DMA pathologically slowly; contiguous quarters stream at full rate).
  - DMA priority: wv, v quarters, wq, q quarter 0, wk, k, q quarters 1-3, wo.
    vh chains pipeline with the v quarters; attention starts as soon as k
    lands (~46us) instead of after the whole input set.
  - a block of dependency-free junk matmuls at the very start holds the PE's
    HAM activity window busy so the real chains run at 2.4 GHz, not the cold
    1.2 GHz default.
  - attention per head-PAIR (2g, 2g+1) per 512-query chunk: row-group
    concurrent K=64 scoresT matmuls for both heads into one PSUM tile, one
    Exp over both (scale 1/8 pre-folded into Wq on host), then M=65 AV
    matmuls whose ones-column accumulates the softmax denominator in row 64.
  - ScalarE runs ONLY the Exps in steady state (the 1104ns/kt pacer); all
    projection PSUM evacuations ride DVE (tensor_scalar_add with the bias
    column).  The two head m=0 evacs use the then-idle ScalarE.
  - division: AV promptly evacuated PSUM->SBUF (frees the accumulator),
    denominator row DMA round-trips through DRAM to broadcast across
    partitions, DVE fast-reciprocal + multiply into attn_outT -- exactly the
    lhsT needed for the out-projection po (q x E) = attn_outT^T @ Wo_loc.
    The final (g3,qc3) division runs in two column halves so the last
    out-projection can start earlier, with junk warm-keeper matmuls holding
    the PE clock up through the division latency.
  - engines run their streams in order, so projection m-tile g+1 matmuls are
    explicitly interleaved into attention pair g's steps (and out-projection
    into pair 3's) to keep TensorE busy while ScalarE paces the exps.
  - output stored fp16 (host sums the two head-half partials in fp32).
"""

import math
import sys
from contextlib import ExitStack

sys.path.insert(0, "/opt/trn_rl_repo")

import numpy as np
import ml_dtypes

import concourse.bass as bass
from concourse import bacc
import concourse.mybir as mybir
import concourse.tile as tile

F32 = mybir.dt.float32
F16 = mybir.dt.float16
BF16 = mybir.dt.bfloat16
AF = mybir.ActivationFunctionType
ALU = mybir.AluOpType

B, S, E = 4, 2048, 1024
H, HD = 16, 64
HLOC = 8          # heads per core
ILOC = HLOC * HD  # 512 internal dims per core
KT = E // 128     # 8 embed k-tiles
ST = S // 128     # 16 seq tiles
NCORES = 8
SCALE = 1.0 / math.sqrt(HD)  # 1/8


def build_nc():
    nc = bacc.Bacc()

    vq_d = [nc.declare_dram_parameter(f"vq{j}", [E, 512], BF16,
                                      isOutput=False).ap() for j in range(4)]
    qq_d = [nc.declare_dram_parameter(f"qq{j}", [E, 512], BF16,
                                      isOutput=False).ap() for j in range(4)]
    kT_d = nc.declare_dram_parameter("kT", [E, S], BF16, isOutput=False).ap()
    wq_d = nc.declare_dram_parameter("wq", [E, ILOC], BF16, isOutput=False).ap()
    wk_d = nc.declare_dram_parameter("wk", [E, ILOC], BF16, isOutput=False).ap()
    wv_d = nc.declare_dram_parameter("wv", [E, ILOC], BF16, isOutput=False).ap()
    wo_d = nc.declare_dram_parameter("wo", [ILOC, E], BF16, isOutput=False).ap()
    bq_d = nc.declare_dram_parameter("bq", [128, 4], F32, isOutput=False).ap()
    bk_d = nc.declare_dram_parameter("bk", [128, 4], F32, isOutput=False).ap()
    out_d = nc.declare_dram_parameter("out", [S, E], F16, isOutput=True).ap()
    dscr = nc.dram_tensor("dscratch", [48, 512], F32).ap()

    with tile.TileContext(nc) as tc, ExitStack() as ctx:
        # ---- pools (PSUM: pp 2x1 + sc 2x2 + av 2x1 = 8 banks) ----
        psum = ctx.enter_context(tc.tile_pool(name="psum", bufs=2, space="PSUM"))
        av_pool = ctx.enter_context(tc.tile_pool(name="avp", bufs=2, space="PSUM"))
        qhT_pool = ctx.enter_context(tc.tile_pool(name="qhT", bufs=2))
        khT_pool = ctx.enter_context(tc.tile_pool(name="khT", bufs=2))
        vh_pool = ctx.enter_context(tc.tile_pool(name="vh", bufs=ST))
        bias_pool = ctx.enter_context(tc.tile_pool(name="bias", bufs=1))
        wpool = ctx.enter_context(tc.tile_pool(name="w_in", bufs=4))
        # svq: v quarters [128, KT, 512]; the q quarters 1-3 reuse the same
        # 4-slot ring once vh has consumed the corresponding v quarter.
        svq_pool = ctx.enter_context(tc.tile_pool(name="svq", bufs=4))
        stage_pool = ctx.enter_context(tc.tile_pool(name="stage", bufs=8))
        exp_pool = ctx.enter_context(tc.tile_pool(name="exp", bufs=4))
        attnT_pool = ctx.enter_context(tc.tile_pool(name="attnT", bufs=4))
        small_pool = ctx.enter_context(tc.tile_pool(name="small", bufs=2))
        bc_pool = ctx.enter_context(tc.tile_pool(name="bcb", bufs=2))
        tmp_pool = ctx.enter_context(tc.tile_pool(name="tmpp", bufs=1))
        out_pool = ctx.enter_context(tc.tile_pool(name="outbuf", bufs=2))

        # qhT/khT for head-pair m are only live while pair m-1/m run, so a
        # 2-slot ring suffices; tiles are created lazily in emission order
        # (m projected as fillers during pair m-1).
        qhT_tiles, khT_tiles = {}, {}

        def qhT(m):
            if m not in qhT_tiles:
                qhT_tiles[m] = qhT_pool.tile(
                    [128, S], BF16, tag="qhT", name=f"qhT{m}")
            return qhT_tiles[m]

        def khT(m):
            if m not in khT_tiles:
                khT_tiles[m] = khT_pool.tile(
                    [128, S], BF16, tag="khT", name=f"khT{m}")
            return khT_tiles[m]

        vh = [vh_pool.tile([128, HLOC * 65], BF16, tag="vh", name=f"vh{i}")
              for i in range(ST)]

        bq_t = bias_pool.tile([128, 4], F32, tag="bq")
        bk_t = bias_pool.tile([128, 4], F32, tag="bk")
        nc.sync.dma_start(bq_t[:], bq_d[:])
        nc.sync.dma_start(bk_t[:], bk_d[:])

        wq_t = wpool.tile([128, KT, ILOC], BF16, tag="w")
        wk_t = wpool.tile([128, KT, ILOC], BF16, tag="w")
        wv_t = wpool.tile([128, KT, ILOC], BF16, tag="w")
        wo_t = wpool.tile([128, 4, E], BF16, tag="w")
        sq0_t = wpool.tile([128, KT, 512], BF16, tag="sq0")
        warm_src = wpool.tile([128, 512], BF16, tag="warm")
        attnT = [attnT_pool.tile([128, S], BF16, tag="attnT",
                                 name=f"attnT{i}") for i in range(4)]

        # ---- stage loads in DMA priority order ----
        nc.sync.dma_start(wv_t[:], wv_d.rearrange("(k p) n -> p k n", p=128))
        svq = []
        for j in range(4):
            t = svq_pool.tile([128, KT, 512], BF16, tag="svq", name=f"svq{j}")
            nc.sync.dma_start(t[:], vq_d[j].rearrange("(k p) n -> p k n",
                                                      p=128))
            svq.append(t)
        nc.sync.dma_start(wq_t[:], wq_d.rearrange("(k p) n -> p k n", p=128))
        nc.sync.dma_start(sq0_t[:], qq_d[0].rearrange("(k p) n -> p k n",
                                                      p=128))
        nc.sync.dma_start(wk_t[:], wk_d.rearrange("(k p) n -> p k n", p=128))
        stg_k = []
        for kk in range(KT):
            t = stage_pool.tile([128, S], BF16, tag="stage", name=f"sk{kk}")
            nc.sync.dma_start(t[:], kT_d[kk * 128:(kk + 1) * 128, :])
            stg_k.append(t)
        sq123 = []
        for j in range(1, 4):
            t = svq_pool.tile([128, KT, 512], BF16, tag="svq", name=f"sq{j}")
            nc.sync.dma_start(t[:], qq_d[j].rearrange("(k p) n -> p k n",
                                                      p=128))
            sq123.append(t)
        nc.sync.dma_start(wo_t[:], wo_d.rearrange("(i p) n -> p i n", p=128))

        # ---- PE warm-up: junk matmuls with no data deps hold the HAM
        # activity window busy so the projection chains run at 2.4 GHz ----
        nc.vector.memset(warm_src[:], 0.0)
        for _ in range(32):
            wps = psum.tile([128, 512], F32, tag="pp", name="wps")
            nc.tensor.matmul(wps[:], lhsT=warm_src[:, 0:128],
                             rhs=warm_src[:], start=True, stop=True)

        def vh_chain(st):
            ps = psum.tile([128, 512], F32, tag="pp", name="psv")
            src = svq[st // 4]
            cs = (st % 4) * 128
            for kk in range(KT):
                nc.tensor.matmul(
                    ps[:],
                    lhsT=src[:, kk, cs:cs + 128],
                    rhs=wv_t[:, kk, :],
                    start=(kk == 0), stop=(kk == KT - 1),
                )
            pin = ps[:].rearrange("p (a b x) -> p a b x", b=2, x=64)
            pout = vh[st].rearrange("p (a c) -> p a c", c=130)
            nc.vector.tensor_copy(pout[:, :, 0:64], pin[:, :, 0, :])
            nc.vector.tensor_copy(pout[:, :, 65:129], pin[:, :, 1, :])
            ones = vh[st].rearrange("p (h x) -> p h x", x=65)[:, :, 64:65]
            nc.vector.memset(ones, 1.0)

        def k_src(kk, quarter):
            return stg_k[kk][:, quarter * 512:(quarter + 1) * 512]

        def q_src(kk, quarter):
            if quarter == 0:
                return sq0_t[:, kk, :]
            return sq123[quarter - 1][:, kk, :]

        def proj_ops(m, which="kq", quarters=range(4), evac_scalar=False):
            """Closure list projecting khT/qhT m-tile chains."""
            ops = []
            srcs = {"k": ((k_src, wk_t, khT, bk_t),),
                    "q": ((q_src, wq_t, qhT, bq_t),)}
            chosen = srcs["k"] + srcs["q"] if which == "kq" else srcs[which]
            for src_fn, w_t, dst, b_t in chosen:
                for quarter in quarters:
                    cols = slice(quarter * 512, (quarter + 1) * 512)
                    holder = {}

                    for kk in range(KT):
                        def mm(kk=kk, src_fn=src_fn, w_t=w_t, quarter=quarter,
                               holder=holder, first=(kk == 0)):
                            if first:
                                holder["ps"] = psum.tile(
                                    [128, 512], F32, tag="pp", name="psqk")
                            nc.tensor.matmul(
                                holder["ps"][:],
                                lhsT=w_t[:, kk, m * 128:(m + 1) * 128],
                                rhs=src_fn(kk, quarter),
                                start=(kk == 0), stop=(kk == KT - 1),
                            )
                        ops.append(mm)

                    def evac(dst=dst, cols=cols, b_t=b_t, m=m, holder=holder,
                             evac_scalar=evac_scalar):
                        if evac_scalar:
                            nc.scalar.activation(
                                dst(m)[:, cols], holder["ps"][:], AF.Identity,
                                bias=b_t[:, m:m + 1],
                            )
                        else:
                            nc.vector.tensor_scalar_add(
                                dst(m)[:, cols], holder["ps"][:],
                                b_t[:, m:m + 1],
                            )
                    ops.append(evac)
            return ops

        def outproj_ops(qc):
            """Closure list projecting output for query chunk qc."""
            ops = []
            for qt in range(qc * 4, qc * 4 + 4):
                holder = {}

                for c in range(2):
                    for it in range(4):
                        def mm(qt=qt, c=c, it=it, holder=holder,
                               first=(c == 0 and it == 0)):
                            if first:
                                holder["ot"] = out_pool.tile(
                                    [128, 1024], F16, tag="ot", name="ot")
                            if it == 0:
                                holder["po"] = psum.tile(
                                    [128, 512], F32, tag="pp", name="po")
                            nc.tensor.matmul(
                                holder["po"][:],
                                lhsT=attnT[it][:, qt * 128:(qt + 1) * 128],
                                rhs=wo_t[:, it, c * 512:(c + 1) * 512],
                                start=(it == 0), stop=(it == 3),
                            )
                        ops.append(mm)

                    def evac(qt=qt, c=c, holder=holder, last=(c == 1)):
                        nc.vector.tensor_copy(
                            holder["ot"][:, c * 512:(c + 1) * 512],
                            holder["po"][:])
                        if last:
                            nc.sync.dma_start(
                                out_d[qt * 128:(qt + 1) * 128, :],
                                holder["ot"][:])
                    ops.append(evac)
            return ops

        def division(g, qc, avA, avB, halves=1):
            """Softmax division for one (g, qc): evacuate av, broadcast the
            denominator row via DRAM, reciprocal + multiply into attnT."""
            qcols0 = qc * 512
            avsA = small_pool.tile([65, 512], F32, tag="avs", name="avsA")
            avsB = small_pool.tile([65, 512], F32, tag="avs", name="avsB")
            nc.vector.tensor_copy(avsA[:], avA[:])
            nc.vector.tensor_copy(avsB[:], avB[:])
            w = 512 // halves
            for h in range(halves):
                cs = h * w
                idx = 32 + 2 * h if halves > 1 else (g * 4 + qc) * 2
                # alternate queues so the two halves' DMA chains issue in
                # parallel on the final (critical-path) division
                dq = nc.gpsimd if h % 2 == 0 else nc.sync
                dq2 = nc.sync if h % 2 == 0 else nc.gpsimd
                dq.dma_start(dscr[idx:idx + 1, 0:w], avsA[64:65, cs:cs + w])
                dq.dma_start(dscr[idx + 1:idx + 2, 0:w],
                             avsB[64:65, cs:cs + w])
                bcA = bc_pool.tile([64, 512], F32, tag="bc", name="bcA")
                bcB = bc_pool.tile([64, 512], F32, tag="bc", name="bcB")
                dq.dma_start(
                    bcA[:, 0:w].rearrange("p (o n) -> p o n", o=1),
                    dscr[idx, 0:w].partition_broadcast(64))
                dq.dma_start(
                    bcB[:, 0:w].rearrange("p (o n) -> p o n", o=1),
                    dscr[idx + 1, 0:w].partition_broadcast(64))
                nc.vector.reciprocal_approx_fast(bcA[:, 0:w], bcA[:, 0:w])
                nc.vector.reciprocal_approx_fast(bcB[:, 0:w], bcB[:, 0:w])
                nc.vector.tensor_mul(
                    attnT[g][0:64, qcols0 + cs:qcols0 + cs + w],
                    avsA[0:64, cs:cs + w], bcA[:, 0:w])
                tmp = tmp_pool.tile([64, 512], BF16, tag="tmp", name="tmp")
                nc.vector.tensor_mul(tmp[:, 0:w],
                                     avsB[0:64, cs:cs + w], bcB[:, 0:w])
                dq2.dma_start(
                    attnT[g][64:128, qcols0 + cs:qcols0 + cs + w],
                    tmp[:, 0:w])

        # ---- head: vh chains pipeline with the v-quarter DMAs, then the
        # m=0 chunk-0 projections (evacs on the then-idle ScalarE) ----
        for st in range(ST):
            vh_chain(st)
        for op in proj_ops(0, "q", quarters=(0,), evac_scalar=True):
            op()
        for op in proj_ops(0, "k", quarters=(0,), evac_scalar=True):
            op()

        for g in range(4):              # head pair (2g, 2g+1)
            hA, hB = 2 * g, 2 * g + 1
            if g == 0:
                fillers = (proj_ops(0, "k", quarters=(1, 2, 3))
                           + proj_ops(0, "q", quarters=(1, 2, 3))
                           + proj_ops(1))
            elif g < 3:
                fillers = proj_ops(g + 1)
            else:
                fillers = []
            steps_left = 4 * ST
            for qc in range(4):         # 512-query chunks
                if g == 3 and qc >= 1:
                    fillers.extend(outproj_ops(qc - 1))
                qcols = slice(qc * 512, (qc + 1) * 512)
                avA = av_pool.tile([65, 512], F32, tag="av", name="avA")
                avB = av_pool.tile([65, 512], F32, tag="av", name="avB")
                for kt in range(ST):
                    sc = psum.tile([128, 1024], F32, tag="sc", name="sc")
                    nc.tensor.matmul(
                        sc[:, 0:512],
                        lhsT=khT(g)[0:64, kt * 128:(kt + 1) * 128],
                        rhs=qhT(g)[0:64, qcols],
                        start=True, stop=True,
                    )
                    nc.tensor.matmul(
                        sc[:, 512:1024],
                        lhsT=khT(g)[64:128, kt * 128:(kt + 1) * 128],
                        rhs=qhT(g)[64:128, qcols],
                        start=True, stop=True,
                    )
                    ex = exp_pool.tile([128, 1024], BF16, tag="exp", name="ex")
                    nc.scalar.activation(ex[:], sc[:], AF.Exp)
                    first, last = (kt == 0), (kt == ST - 1)
                    nc.tensor.matmul(
                        avA[0:65, :],
                        lhsT=vh[kt][:, hA * 65:hA * 65 + 65],
                        rhs=ex[:, 0:512],
                        start=first, stop=last,
                    )
                    nc.tensor.matmul(
                        avB[0:65, :],
                        lhsT=vh[kt][:, hB * 65:hB * 65 + 65],
                        rhs=ex[:, 512:1024],
                        start=first, stop=last,
                    )
                    # pace interleaved filler work (proj / out-proj);
                    # front-load during (g0,qc0) so khT/qhT m=0 quarters
                    # finish before the kt sweep reaches them
                    steps_left -= 1
                    n_take = -(-len(fillers) // max(steps_left, 1)) \
                        if fillers else 0
                    if g == 0 and qc == 0 and fillers:
                        n_take = max(n_take, 3)
                    for _ in range(min(n_take, len(fillers))):
                        fillers.pop(0)()
                if g == 3 and qc == 3:
                    # hold the PE warm through the final division latency
                    for _ in range(20):
                        wps = psum.tile([128, 512], F32, tag="pp", name="wk2")
                        nc.tensor.matmul(wps[:], lhsT=warm_src[:, 0:128],
                                         rhs=warm_src[:], start=True,
                                         stop=True)
                    division(g, qc, avA, avB, halves=2)
                else:
                    division(g, qc, avA, avB)
            # flush any leftover fillers for this pair
            for op in fillers:
                op()

        # final out-projection chunk
        for op in outproj_ops(3):
            op()

    nc.finalize()
    return nc


def make_in_maps(q, k, v, Wq, bq, Wk, bk, Wv, bv, Wo, bo):
    """Per-core input dicts + the folded host-side bias."""
    bf = ml_dtypes.bfloat16
    qT = [np.ascontiguousarray(q[b].T).astype(bf) for b in range(B)]
    kT = [np.ascontiguousarray(k[b].T).astype(bf) for b in range(B)]
    vT = [np.ascontiguousarray(v[b].T).astype(bf) for b in range(B)]
    in_maps = []
    for c in range(NCORES):
        b, hh = divmod(c, 2)
        isl = slice(hh * ILOC, (hh + 1) * ILOC)
        bq_loc = np.ascontiguousarray(
            (bq[isl] * SCALE).reshape(4, 128).T).astype(np.float32)
        bk_loc = np.ascontiguousarray(
            bk[isl].reshape(4, 128).T).astype(np.float32)
        m = {
            "kT": kT[b],
            "wq": np.ascontiguousarray(Wq[:, isl] * SCALE).astype(bf),
            "wk": np.ascontiguousarray(Wk[:, isl]).astype(bf),
            "wv": np.ascontiguousarray(Wv[:, isl]).astype(bf),
            "wo": np.ascontiguousarray(Wo[isl, :]).astype(bf),
            "bq": bq_loc, "bk": bk_loc,
        }
        for j in range(4):
            m[f"vq{j}"] = np.ascontiguousarray(vT[b][:, j * 512:(j + 1) * 512])
            m[f"qq{j}"] = np.ascontiguousarray(qT[b][:, j * 512:(j + 1) * 512])
        in_maps.append(m)
    bo_eff = (bo + bv @ Wo).astype(np.float32)
    return in_maps, bo_eff


_NC_CACHE = None


def kernel(q, k, v, Wq, bq, Wk, bk, Wv, bv, Wo, bo):
    global _NC_CACHE
    from concourse.bass_utils import run_bass_kernel_spmd

    if _NC_CACHE is None:
        _NC_CACHE = build_nc()
    nc = _NC_CACHE
    in_maps, bo_eff = make_in_maps(q, k, v, Wq, bq, Wk, bk, Wv, bv, Wo, bo)
    res = run_bass_kernel_spmd(nc, in_maps, list(range(NCORES)))
    out = np.empty((B, S, E), np.float32)
    for b in range(B):
        out[b] = (res.results[2 * b]["out"].astype(np.float32)
                  + res.results[2 * b + 1]["out"].astype(np.float32)
                  + bo_eff)
    return out
